# revision 7
# baseline (speedup 1.0000x reference)
"""Causal multi-head attention on 8 Trainium2 NeuronCores.

Problem (fp32): x [2,2048,1024]; Wq/Wk/Wv/Wo [1024,1024] (+biases);
16 heads x 64 dims; causal softmax attention.  ~68.7 GFLOP total.

Sharding: core c handles batch b = c//4 and head group g = c%4
(4 heads = 256 of the 1024 qkv dims).  Each core computes its partial
out = attn_heads(b, g) @ Wo[256 rows] as fp16 and the host sums the 4
partials per batch.  bq/bk applied on-device (they affect softmax); bv
on-device in the V projection; bo once on the host.

Device algorithm per core (transposed flash attention; scores are
~N(0,1) so fp32-accumulated exp needs no running-max subtraction).
All matmul operands are fp16 (1 cyc/row on the PE at any N; fp32
accumulation in PSUM):
  - all inputs are HOST-PRE-ARRANGED into their exact SBUF layouts so
    every DMA is a plain [128, L] copy with 1-8KB contiguous lines
    (the DMA ring processes ~1 packet/57ns/engine, so packet size —
    not bytes — is what limits the initial load; scattered 256B-line
    rearranges previously cost an 18us DMA-bound head)
  - Q^T/K^T = W.T @ x.T with W chunks stationary (d on partitions);
    Q^T lands per-head with the sibling head's 64 partitions zeroed so
    scores contract the full 128 partitions of K^T
  - V = x @ Wv with x^T chunks stationary; the [V|1] stationary for
    the PV matmul carries 64 all-ones columns, so PSUM rows 64:128 of
    the out^T accumulator come out holding the softmax denominator
    REPLICATED across 64 partitions — the partition broadcast of 1/s
    costs nothing (no DRAM bounce, no K=1 matmul, no [1,512] ACT ops)
  - scores^T[k,q] = K^T_chunk.T @ Q^T, two chunks paired into one
    2-bank PSUM tile so one ACT exp covers 1024 columns (ACT has
    ~220ns/instr overhead; pairing + causal trimming cuts ACT from
    ~123us to ~82us, below the PE's ~125us).  Diagonal chunks keep
    per-chunk exps trimmed to their live columns (m=0:512, m=1:384,
    m=2:256, m=3:128 — fp16 has no N>=256 rate cliff) with 0/1 mask
    multiplies on DVE
  - normalization: rec = reciprocal_approx_fast(po[64:128]) on DVE
    (~51 ULP), then at = po[0:64] * rec -> fp16
  - final partial = at.T @ Wo chunks -> fp16 out.  On the last block
    the at[0] (heads 0,1) half of the contraction is emitted inside
    head 2's attention and accumulated in SBUF, so only the at[1]
    matmuls + adds + one DMA trail the last PV matmul
  - PSUM (8 banks): pa = QK-proj/out-proj shared pool (2), pb =
    V-proj/out^T-accum shared pool (2), ps = paired scores (2x2).
    The phases sharing a pool never overlap by more than the pool
    depth, and cross-phase slot reuse only adds already-satisfied deps

The _split_sync_waits post-pass works around the installed walrus
accepting only one sync wait command per instruction.
"""

import numpy as np

B, S, D = 2, 2048, 1024
H, DK, DV = 16, 64, 64
D_OUT = 1024
N_CORES = 8
H_LOC = H // 4          # 4 heads per core
DLOC = H_LOC * DK       # 256 qkv dims per core
NBLK = S // 512         # 4 query blocks of 512 tokens
NKB = S // 128          # 16 key chunks of 128 tokens

DT_MM_NAME = "float16"

_CACHE = {}

# q0(m): first live query column of diagonal chunk m (columns below it
# are fully causally masked)
_Q0 = {0: 0, 1: 128, 2: 256, 3: 384}


def _build_nc():
    import bass_rust
    import concourse.bass as bass
    import concourse.mybir as mybir
    import concourse.tile as tile

    FP = mybir.dt.float32
    DT = getattr(mybir.dt, DT_MM_NAME)

    def _split_sync_waits(nc_):
        """The installed walrus accepts only ONE sync wait command per
        instruction; Tile emits several (worst on the exit drain). Hoist
        extra waits onto nop instructions inserted just before, on the
        same engine queue — in-order queue execution keeps semantics."""
        n = 0
        for f in nc_.m.functions:
            for bb in f.blocks:
                out = []
                for inst in bb.instructions:
                    si = inst.sync_info
                    waits = list(si.on_wait) if si and si.on_wait else []
                    if len(waits) > 1:
                        for w in waits[:-1]:
                            n += 1
                            nop = mybir.InstNoOp(
                                name=f"{inst.name}-wsplit{n}",
                                sync_info=bass_rust.SyncInfo(
                                    on_wait=[w], on_update=[]
                                ),
                                bass_nofuse=True,
                                engine=inst.engine,
                            )
                            nc_.register_instruction(nop, overwrite=True)
                            out.append(nop)
                        inst.sync_info = bass_rust.SyncInfo(
                            on_wait=waits[-1:], on_update=list(si.on_update or [])
                        )
                    out.append(inst)
                bb.instructions[:] = out

    nc = bass.Bass(target_bir_lowering=False)
    nc._allow_low_precision_reason = "fp16 matmul inputs, fp32 PSUM accum"

    xs_d = nc.dram_tensor("xs", [128, NBLK, 8, 512], DT, kind="ExternalInput")
    wq_d = nc.dram_tensor("wq", [128, 8, DLOC], DT, kind="ExternalInput")
    wk_d = nc.dram_tensor("wk", [128, 8, DLOC], DT, kind="ExternalInput")
    wv_d = nc.dram_tensor("wv", [128, 8, DLOC], DT, kind="ExternalInput")
    wo_d = nc.dram_tensor("wo", [128, 2, D_OUT], DT, kind="ExternalInput")
    bqk_d = nc.dram_tensor("bqk", [128, 4], FP, kind="ExternalInput")
    bvb_d = nc.dram_tensor("bvb", [128, DLOC], DT, kind="ExternalInput")
    msk_d = nc.dram_tensor("msk", [128, 4, 512], DT, kind="ExternalInput")
    out_d = nc.dram_tensor("out", [S, D_OUT], DT, kind="ExternalOutput")

    Exp = mybir.ActivationFunctionType.Exp
    Ln = mybir.ActivationFunctionType.Ln

    with tile.TileContext(nc) as tc:
        from contextlib import ExitStack

        stack = ExitStack()
        with stack:
            cpool = stack.enter_context(tc.tile_pool(name="consts", bufs=1))
            ppool = stack.enter_context(tc.tile_pool(name="persist", bufs=1))
            xtpool = stack.enter_context(tc.tile_pool(name="xt", bufs=2))
            qtpool = stack.enter_context(tc.tile_pool(name="qt", bufs=2))
            atpool = stack.enter_context(tc.tile_pool(name="at", bufs=2))
            espool = stack.enter_context(tc.tile_pool(name="es", bufs=4))
            rpool = stack.enter_context(tc.tile_pool(name="rec", bufs=2))
            opool = stack.enter_context(tc.tile_pool(name="outs", bufs=4))
            papool = stack.enter_context(tc.tile_pool(name="pa", bufs=2, space="PSUM"))
            popool = stack.enter_context(tc.tile_pool(name="po", bufs=2, space="PSUM"))
            pspool = stack.enter_context(tc.tile_pool(name="ps", bufs=4, space="PSUM"))

            # ---- constants / persistent ----
            wq_sb = cpool.tile([128, 8, DLOC], DT)
            wk_sb = cpool.tile([128, 8, DLOC], DT)
            wv_sb = cpool.tile([128, 8, DLOC], DT)
            wo_sb = cpool.tile([128, 2, D_OUT], DT)
            bqk_sb = cpool.tile([128, 4], FP)
            bvb_sb = cpool.tile([128, DLOC], DT)
            msk_sb = cpool.tile([128, 4, 512], DT)
            kt_sb = [ppool.tile([128, S], DT, name=f"kt{i}") for i in range(2)]
            # [V | ones] stationaries: per chunk, per head, cols 0:64 = V,
            # cols 64:128 = 1.0 (denominator-broadcast trick)
            vsb = ppool.tile([128, NKB, 4, 128], DT)
            nc.vector.memset(vsb[:, :, :, 64:128], 1.0)

            # ---- initial DMAs, interleaved so the first QK-projection
            # matmuls start ~1.5us after the first packet ----
            xt_tiles = {}

            def issue_xt(jb):
                xt = xtpool.tile([128, 8, 512], DT, name=f"xt{jb}")
                for half in range(2):
                    cs = slice(4 * half, 4 * (half + 1))
                    nc.sync.dma_start(xt[:, cs, :], xs_d[:, jb, cs, :])
                xt_tiles[jb] = xt

            nc.sync.dma_start(wq_sb[:, 0:4, :], wq_d[:, 0:4, :])
            issue_xt(0)
            nc.sync.dma_start(wq_sb[:, 4:8, :], wq_d[:, 4:8, :])
            nc.sync.dma_start(wk_sb[:, 0:4, :], wk_d[:, 0:4, :])
            nc.sync.dma_start(wk_sb[:, 4:8, :], wk_d[:, 4:8, :])
            nc.sync.dma_start(bqk_sb[:], bqk_d[:])
            nc.sync.dma_start(bvb_sb[:], bvb_d[:])
            nc.sync.dma_start(wv_sb[:], wv_d[:])
            nc.sync.dma_start(msk_sb[:], msk_d[:])

            for jb in range(NBLK):
                tok0 = jb * 512
                last = jb == NBLK - 1
                xt = xt_tiles[jb]

                # ---- Q^T / K^T projections ----
                qz = [qtpool.tile([128, 512], DT, name=f"qz{i}") for i in range(4)]
                for wsb, bcol in ((wq_sb, 0), (wk_sb, 2)):
                    for mt in range(2):
                        pq = papool.tile([128, 512], FP, name="pq", tag="pa")
                        for c in range(8):
                            nc.tensor.matmul(
                                pq[:],
                                wsb[:, c, 128 * mt : 128 * (mt + 1)],
                                xt[:, c, :],
                                start=(c == 0),
                                stop=(c == 7),
                            )
                        if bcol == 0:
                            for hp in range(2):
                                own = slice(64 * hp, 64 * hp + 64)
                                oth = slice(64 * (1 - hp), 64 * (1 - hp) + 64)
                                qzh = qz[2 * mt + hp]
                                if jb < 2:
                                    # slots cycle with bufs=2; later blocks
                                    # inherit these zeros untouched
                                    nc.vector.memset(qzh[oth, :], 0.0)
                                nc.vector.tensor_scalar_add(
                                    qzh[own, :],
                                    pq[own, :],
                                    bqk_sb[own, mt : mt + 1],
                                )
                        else:
                            nc.vector.tensor_scalar_add(
                                kt_sb[mt][:, tok0 : tok0 + 512],
                                pq[:],
                                bqk_sb[:, bcol + mt : bcol + mt + 1],
                            )

                # ---- V projection for this block's 4 key chunks ----
                for t in range(4):
                    kb = jb * 4 + t
                    pv = papool.tile([128, DLOC], FP, name="pv", tag="pa")
                    for c in range(8):
                        nc.tensor.matmul(
                            pv[:],
                            xt[:, c, 128 * t : 128 * (t + 1)],
                            wv_sb[:, c, :],
                            start=(c == 0),
                            stop=(c == 7),
                        )
                    nc.vector.tensor_add(
                        vsb[:, kb, :, 0:64],
                        pv[:].rearrange("p (h w) -> p h w", h=4),
                        bvb_sb[:].rearrange("p (h w) -> p h w", h=4),
                    )

                # prefetch next block's x^T; first-block extras ride after
                if jb + 1 < NBLK:
                    issue_xt(jb + 1)
                if jb == 0:
                    nc.sync.dma_start(wo_sb[:], wo_d[:])

                # ---- attention for this query block ----
                at = [atpool.tile([128, 512], DT, name=f"at{i}") for i in range(2)]
                nkc = 4 * (jb + 1)

                def emit_outproj_half(vc, accumulate):
                    """One half (at[vc] against wo[:, vc]) of the output
                    projection for every (qc, dblk).  accumulate=False
                    copies PSUM->o_sb; True adds into o_sb."""
                    for qc in range(4):
                        for dblk in range(2):
                            pf = pspool.tile([128, 512], FP, name="pf", tag="ps")
                            nc.tensor.matmul(
                                pf[:],
                                at[vc][:, 128 * qc : 128 * (qc + 1)],
                                wo_sb[:, vc, 512 * dblk : 512 * (dblk + 1)],
                                start=True,
                                stop=True,
                            )
                            dst = o_tiles[qc][:, 512 * dblk : 512 * (dblk + 1)]
                            if accumulate:
                                nc.vector.tensor_add(dst, dst, pf[:])
                            else:
                                nc.vector.tensor_copy(dst, pf[:])

                o_tiles = [
                    opool.tile([128, D_OUT], DT, name=f"o{qc}") for qc in range(4)
                ]

                for h in range(4):
                    p0 = 64 * (h % 2)
                    qt_h = qz[h]
                    kt_h = kt_sb[h // 2]
                    po = popool.tile([128, 512], FP, name="po")
                    for kc in range(nkc):
                        m = kc - 4 * jb
                        q0 = _Q0.get(m, 0)
                        ps = pspool.tile([128, 512], FP, name="ps", tag="ps")
                        nc.tensor.matmul(
                            ps[:, q0:512],
                            kt_h[:, 128 * kc : 128 * (kc + 1)],
                            qt_h[:, q0:512],
                            start=True,
                            stop=True,
                        )
                        es = espool.tile([128, 512], DT)
                        nc.scalar.activation(
                            es[:, q0:512], ps[:, q0:512], Exp, scale=0.125
                        )
                        if m >= 0:
                            nc.vector.tensor_mul(
                                es[:, q0:512],
                                es[:, q0:512],
                                msk_sb[:, m, q0:512],
                            )
                        nc.tensor.matmul(
                            po[:, q0:512],
                            vsb[:, kc, h, :],
                            es[:, q0:512],
                            start=(kc == 0),
                            stop=(kc == nkc - 1),
                        )
                        # on the last block, heads 0,1 are final once their
                        # norm lands; emit their half of the output
                        # projection inside head 2's stream so only the
                        # at[1] half trails the last PV matmul
                        if last and h == 2 and kc == 7:
                            emit_outproj_half(0, accumulate=False)
                    # normalization: denominator sits replicated on PSUM
                    # rows 64:128 (ones-columns trick) — 1/s = exp(-ln s)
                    # on ACT (walrus rejects the custom-DVE fast reciprocal
                    # and the DVE divide op; plain DVE reciprocal is
                    # ~3.4us), then one DVE multiply; no PE or partition
                    # broadcast involved
                    with tc.high_priority():
                        lns = rpool.tile([64, 512], FP, name="lns")
                        rec = rpool.tile([64, 512], FP, name="rec")
                        nc.scalar.activation(lns[:], po[64:128, :], Ln)
                        nc.scalar.activation(rec[:], lns[:], Exp, scale=-1.0)
                        nc.vector.tensor_mul(
                            at[h // 2][p0 : p0 + 64, :], po[0:64, :], rec[:]
                        )

                # ---- output projection ----
                if last:
                    emit_outproj_half(1, accumulate=True)
                else:
                    for qc in range(4):
                        for dblk in range(2):
                            pf = pspool.tile([128, 512], FP, name="pf", tag="ps")
                            for vc in range(2):
                                nc.tensor.matmul(
                                    pf[:],
                                    at[vc][:, 128 * qc : 128 * (qc + 1)],
                                    wo_sb[:, vc, 512 * dblk : 512 * (dblk + 1)],
                                    start=(vc == 0),
                                    stop=(vc == 1),
                                )
                            nc.vector.tensor_copy(
                                o_tiles[qc][:, 512 * dblk : 512 * (dblk + 1)],
                                pf[:],
                            )
                for qc in range(4):
                    r0 = tok0 + 128 * qc
                    nc.sync.dma_start(out_d[r0 : r0 + 128, :], o_tiles[qc][:])

    _split_sync_waits(nc)
    return nc


def _get_nc():
    if "nc" not in _CACHE:
        _CACHE["nc"] = _build_nc()
    return _CACHE["nc"]


def kernel(x, Wq, bq, Wk, bk, Wv, bv, Wo, bo, _trace=False):
    from concourse.bass_utils import run_bass_kernel_spmd

    if DT_MM_NAME == "bfloat16":
        import ml_dtypes

        np_dt = ml_dtypes.bfloat16
    elif DT_MM_NAME == "float16":
        np_dt = np.float16
    else:
        np_dt = np.float32

    x = np.asarray(x, dtype=np.float32)
    Wq, bq = np.asarray(Wq, np.float32), np.asarray(bq, np.float32)
    Wk, bk = np.asarray(Wk, np.float32), np.asarray(bk, np.float32)
    Wv, bv = np.asarray(Wv, np.float32), np.asarray(bv, np.float32)
    Wo, bo = np.asarray(Wo, np.float32), np.asarray(bo, np.float32)

    # causal 0/1 masks for the 4 diagonal positions of a 512-query block
    p = np.arange(128)[:, None, None]
    m = np.arange(4)[None, :, None]
    q = np.arange(512)[None, None, :]
    msk = (q >= p + 128 * m).astype(np_dt)

    def wlayout(W):  # [1024, 256] -> [128, 8, 256], chunk-major partitions
        return np.ascontiguousarray(
            W.reshape(8, 128, DLOC).transpose(1, 0, 2)
        ).astype(np_dt)

    in_maps = []
    for c in range(N_CORES):
        b, g = c // 4, c % 4
        s = slice(g * DLOC, (g + 1) * DLOC)
        bq_s, bk_s = bq[s], bk[s]
        bqk = np.stack(
            [bq_s[:128], bq_s[128:], bk_s[:128], bk_s[128:]], axis=1
        ).astype(np.float32)
        xs = np.ascontiguousarray(
            x[b].reshape(NBLK, 512, 8, 128).transpose(3, 0, 2, 1)
        ).astype(np_dt)
        wo_l = np.ascontiguousarray(
            Wo[s, :].reshape(2, 128, D_OUT).transpose(1, 0, 2)
        ).astype(np_dt)
        in_maps.append(
            {
                "xs": xs,
                "wq": wlayout(Wq[:, s]),
                "wk": wlayout(Wk[:, s]),
                "wv": wlayout(Wv[:, s]),
                "wo": wo_l,
                "bqk": bqk,
                "bvb": np.tile(bv[s][None, :], (128, 1)).astype(np_dt),
                "msk": msk,
            }
        )

    nc = _get_nc()
    res = run_bass_kernel_spmd(nc, in_maps, list(range(N_CORES)), trace=_trace)

    out = np.empty((B, S, D_OUT), dtype=np.float32)
    for b in range(B):
        acc = res.results[4 * b]["out"].astype(np.float32)
        for g in range(1, 4):
            acc = acc + res.results[4 * b + g]["out"].astype(np.float32)
        out[b] = acc + bo[None, :]
    if _trace:
        return out, res
    return out
